# revision 15
# baseline (speedup 1.0000x reference)
"""Trainium2 Bass kernel for a GNN message-passing layer.

Reference computation (all fp32):
    messages = h[src] @ W_msg.T            # [E, D]
    agg      = segment_sum(messages, dst)  # [N, D]
    out      = relu(concat(h, agg) @ W_upd.T + b_upd)

Key algebraic restructure: segment_sum is linear, so
    agg = A @ W_msg.T          where A = segment_sum(h[src], dst)
and the update splits W_upd = [Wu1 | Wu2]:
    out.T = relu(Wu1 @ h.T + (Wu2 @ W_msg) @ A.T + b)
so the device only computes A (a pure gather + scatter-add) plus two small
fused matmuls.  Wc = Wu2 @ W_msg is precomputed on host.

Sharding: nodes are partitioned contiguously across the 8 cores by dst.
Each core processes exactly the edges whose dst lands in its node shard
(host buckets edges by 128-node dst block), so no collectives are needed.

The kernel is SWDGE-descriptor-generation bound: every gathered h row costs
one software-generated DMA descriptor (~2.2 ns, serialized on the GpSimd
engine ucode).  Gathers sized at one (block, parity) (~2.2k descriptors)
fit the SWDGE ring; larger instructions stall on ring reclaim (measured).

Per core, per destination-node block (128 nodes):
  - the block's edges are padded to a fixed number of 128-edge chunks;
    pad slots gather an all-zero row appended to h, so they contribute 0
  - two dma_gather instructions fetch h[src] (bf16) for the block's edges
    (indices are int16, so rows are split even/odd and gathered from
    strided views h[0::2] / h[1::2] with idx = src>>1), spread across 4
    SWDGE queues.  Edges are sorted by src within each bucket for DRAM
    page locality.
  - scatter-add via a 0/1 staircase: ONE VectorE tensor_tensor per block
        S[e, c, jj] = (jj < rel[e,c] + 0.5)        in {0, 1}, jj = 0..129
    batched over all chunks of the block.  relp is stored duplicated
    (each value twice) so every operand's innermost AP dim is
    (stride 1, count 2) 2-byte — this keeps the DVE in its 2x perf mode
    despite the broadcasts.  One TensorE matmul per chunk accumulates
        psum[i, jj] += sum_e g[e, i] * S[e, jj]     (bf16 x bf16 -> fp32)
    The node sums are adjacent-column differences
        A.T[i, b*128 + j] = psum[i, j] - psum[i, j+1]
Phase 2 (per 4-block group): the adjacent-column difference is taken on
VectorE (fp32 psum copies -> bf16), then two bf16 matmuls
    out.T = relu(Wu1 @ h.T + Wc @ diff + b)
run at full PE rate (fp32 matmuls cost 4 cycles/row; bf16 costs 1).
"""

import contextlib

import numpy as np

import concourse.bass as bass
import concourse.mybir as mybir
import concourse.tile as tile
from concourse import bacc
from concourse.bass_utils import run_bass_kernel_spmd

P = 128  # SBUF partitions
D = 128  # feature dim (in_dim == out_dim == 128)
N_CORES = 8
CHUNK = 128  # edges per matmul chunk
W129 = CHUNK + 1  # staircase width per block (psum / buf)
W130 = CHUNK + 2  # staircase width incl. pad col (even for 2x DVE mode)
GAT_BUFS = 5  # gather tile pool depth
WARM = GAT_BUFS  # first blocks gather pads for real (warm every pool buf)
SCRATCH = 16384  # SWDGE descriptor carveout bytes/partition (default 16384)

_prog_cache: dict = {}


def _build_program(
    N: int, SP: int, NB: int, KE: int, KO: int, nreg=None, loop_iters=None
):
    """One SPMD program, shared by all 8 cores.

    N      : rows of the (replicated) h table incl. 2 appended zero rows
    SP     : padded nodes per core (NB * 128)
    NB     : 128-node blocks per core
    KE, KO : 128-edge chunks per block for even-src / odd-src edges
    loop_iters : if set, wrap the compute body in a For_i hardware loop
                 executing it that many times (wall-clock timing harness)
    """
    f32 = mybir.dt.float32
    bf16 = mybir.dt.bfloat16
    i16 = mybir.dt.int16
    NCH = KE + KO
    BCOLS = NCH * 8  # idx int16 columns per block
    if nreg is None:
        nreg = [(KE * CHUNK, KO * CHUNK)] * NB

    nc = bacc.Bacc(
        "TRN2",
        target_bir_lowering=False,
        num_swdge_queues=4,
        dynamic_dma_scratch_size=SCRATCH,
    )

    h_d = nc.dram_tensor("h", [N, D], bf16, kind="ExternalInput")
    hsT_d = nc.dram_tensor("hsT", [P, SP], bf16, kind="ExternalInput")
    idx_d = nc.dram_tensor("idx", [P, NB * BCOLS], i16, kind="ExternalInput")
    relp_d = nc.dram_tensor("relp", [P, NB * NCH * 2], bf16, kind="ExternalInput")
    iota_d = nc.dram_tensor("iota", [P, W130], bf16, kind="ExternalInput")
    w1_d = nc.dram_tensor("w1T", [D, D], bf16, kind="ExternalInput")
    wc_d = nc.dram_tensor("wcT", [D, D], bf16, kind="ExternalInput")
    b_d = nc.dram_tensor("bias", [P, 1], f32, kind="ExternalInput")
    out_d = nc.dram_tensor("outT", [P, SP], f32, kind="ExternalOutput")

    h_even = h_d[0:N:2, :]
    h_odd = h_d[1:N:2, :]
    h_pairs = h_d[:].rearrange("(a b) d -> a (b d)", b=2)

    with tile.TileContext(nc) as tc:
        with (
            tc.tile_pool(name="constp", bufs=1) as constp,
            tc.tile_pool(name="gatp", bufs=5) as gatp,
            tc.tile_pool(name="sp_", bufs=3) as sp_,
            tc.tile_pool(name="aggp", bufs=1) as aggp,
            tc.tile_pool(name="diffp", bufs=2) as diffp,
            tc.tile_pool(name="outp", bufs=3) as outp,
            tc.tile_pool(name="psp", bufs=6, space="PSUM") as psp,
            tc.tile_pool(name="ps2p", bufs=2, space="PSUM") as ps2p,
        ):
            # load order matters: the first gathers wait on iota/idx/relp,
            # so those go first (idx split per block); hsT (phase 2) last
            iota_t = constp.tile([P, W130], bf16)
            nc.sync.dma_start(iota_t[:], iota_d[:])
            # idx loads split 3-way so the first gathers start immediately
            # (a monolithic load costs ~14us of startup; per-block splits
            # put a sem-wait on the serial GpSimd row per gather — worse)
            idx_t = constp.tile([P, NB * BCOLS], i16)
            nc.sync.dma_start(idx_t[:, 0:BCOLS], idx_d[:, 0:BCOLS])
            cut = min(5, NB) * BCOLS
            nc.sync.dma_start(idx_t[:, BCOLS:cut], idx_d[:, BCOLS:cut])
            if cut < NB * BCOLS:
                nc.sync.dma_start(idx_t[:, cut:], idx_d[:, cut:])
            relp_t = constp.tile([P, NB * NCH * 2], bf16)
            nc.sync.dma_start(relp_t[:], relp_d[:])
            w1_t = constp.tile([D, D], bf16)
            nc.sync.dma_start(w1_t[:], w1_d[:])
            wc_t = constp.tile([D, D], bf16)
            nc.sync.dma_start(wc_t[:], wc_d[:])
            b_t = constp.tile([P, 1], f32)
            nc.sync.dma_start(b_t[:], b_d[:])
            hsT_t = constp.tile([P, SP], bf16)
            nc.sync.dma_start(hsT_t[:], hsT_d[:])

            # staircase psum copies: per block 129 columns
            buf_t = aggp.tile([P, NB * W129], f32)

            iota_ab = iota_t[:].rearrange("p (a b) -> p a b", b=2)

            loop_cm = (
                tc.For_i(0, loop_iters, 1)
                if loop_iters is not None
                else contextlib.nullcontext()
            )
            with loop_cm:
                # Phase 1: staircase accumulation per block
                for b in range(NB):
                    g_t = gatp.tile([P, NCH * D], bf16)
                    g3 = g_t[:].rearrange("p (c d) -> p c d", c=NCH)
                    icol = b * BCOLS
                    nc.gpsimd.dma_gather(
                        out_ap=g3[:, 0:KE, :],
                        in_ap=h_even,
                        idxs_ap=idx_t[:, icol : icol + KE * 8],
                        num_idxs=KE * CHUNK,
                        num_idxs_reg=KE * CHUNK,
                        elem_size=D,
                        elem_step=2 * D,
                        single_packet=False,
                        queue_num=(2 * b) % 4,
                    )
                    nc.gpsimd.dma_gather(
                        out_ap=g3[:, KE:NCH, :],
                        in_ap=h_odd,
                        idxs_ap=idx_t[:, icol + KE * 8 : icol + BCOLS],
                        num_idxs=KO * CHUNK,
                        num_idxs_reg=KO * CHUNK,
                        elem_size=D,
                        elem_step=2 * D,
                        single_packet=False,
                        queue_num=(2 * b + 1) % 4,
                    )
                    # ONE DVE op: S[p, c, jj] = (iota[jj] < relp[p, c])
                    s_t = sp_.tile([P, NCH * W130], bf16)
                    relp_b = (
                        relp_t[:, b * NCH * 2 : (b + 1) * NCH * 2]
                        .rearrange("p (c t) -> p c t", t=2)
                        .unsqueeze(2)
                        .broadcast_to([P, NCH, W130 // 2, 2])
                    )
                    iota_b = iota_ab.unsqueeze(1).broadcast_to(
                        [P, NCH, W130 // 2, 2]
                    )
                    s_b = s_t[:].rearrange(
                        "p (c a b) -> p c a b", a=W130 // 2, b=2
                    )
                    nc.vector.tensor_tensor(
                        out=s_b, in0=iota_b, in1=relp_b, op=mybir.AluOpType.is_lt
                    )
                    s3 = s_t[:].rearrange("p (c w) -> p c w", w=W130)
                    ps_t = psp.tile([P, W129], f32)
                    for c in range(NCH):
                        nc.tensor.matmul(
                            out=ps_t[:],
                            lhsT=g_t[:, c * D : (c + 1) * D],
                            rhs=s3[:, c, 0:W129],
                            start=(c == 0),
                            stop=(c == NCH - 1),
                        )
                    nc.scalar.activation(
                        out=buf_t[:, b * W129 : (b + 1) * W129],
                        in_=ps_t[:],
                        func=mybir.ActivationFunctionType.Copy,
                    )

                # Phase 2 per 4-block group:
                #   diff = bufA - bufB (VectorE, fp32 -> bf16)
                #   out.T = relu(Wu1 @ h.T + Wc @ diff + b)   (bf16 matmuls)
                buf3 = buf_t[:].rearrange("p (b j) -> p b j", j=W129)
                b0 = 0
                while b0 < NB:
                    nb = min(4, NB - b0)
                    w = nb * CHUNK
                    col = b0 * CHUNK
                    d_t = diffp.tile([P, 512], bf16)
                    d3 = d_t[:].rearrange("p (b j) -> p b j", j=CHUNK)
                    nc.vector.tensor_tensor(
                        out=d3[:, 0:nb, :],
                        in0=buf3[:, b0 : b0 + nb, 0:CHUNK],
                        in1=buf3[:, b0 : b0 + nb, 1:W129],
                        op=mybir.AluOpType.subtract,
                    )
                    ps2_t = ps2p.tile([P, 512], f32)
                    nc.tensor.matmul(
                        out=ps2_t[:, :w],
                        lhsT=w1_t[:],
                        rhs=hsT_t[:, col : col + w],
                        start=True,
                        stop=False,
                    )
                    nc.tensor.matmul(
                        out=ps2_t[:, :w],
                        lhsT=wc_t[:],
                        rhs=d_t[:, :w],
                        start=False,
                        stop=True,
                    )
                    o_t = outp.tile([P, 512], f32)
                    nc.scalar.activation(
                        o_t[:, :w],
                        ps2_t[:, :w],
                        mybir.ActivationFunctionType.Relu,
                        bias=b_t[:],
                    )
                    nc.sync.dma_start(out_d[:, col : col + w], o_t[:, :w])
                    b0 += nb

    nc.compile()
    return nc


def _prep_inputs(h, edge_index, W_msg, W_upd, b_upd):
    """Host-side sharding: bucket edges by destination-node block, then
    split each block's edges by src parity for the int16 dma_gather.

    Blocks are assigned to (core, slot) by descending edge count: slot s
    holds ranks [8s, 8s+8) spread across the 8 cores, so one SPMD-shared
    num_idxs_reg per (slot, parity) (the max over its 8 blocks) is tight.
    Pad gather slots beyond that count carry idx=-1 and are SKIPPED by the
    SWDGE ucode (no DMA packet).  Slots < WARM instead gather pads for real
    so every gather-pool buffer holds finite bf16 data before any skipped
    (stale-data) tail can appear under a zero staircase row.
    """
    import ml_dtypes

    N0, d = h.shape
    assert d == D
    E = edge_index.shape[1]

    SP = -(-N0 // (N_CORES * P)) * P  # padded nodes per core
    NB = SP // P
    n_blocks_tot = N_CORES * NB

    src = np.ascontiguousarray(edge_index[0]).astype(np.int64)
    dst = np.ascontiguousarray(edge_index[1]).astype(np.int64)

    # h with two appended zero rows (one per parity) for pad-edge gathers
    N = N0 + 2
    hg = np.zeros((N, D), dtype=ml_dtypes.bfloat16)
    hg[:N0] = h.astype(ml_dtypes.bfloat16)
    pad_idx = N0 // 2  # row N0 (even) / N0+1 (odd), both zero

    # order edges by (dst block, src parity), then src for DRAM locality
    gblock = dst >> 7
    parity = src & 1
    order = np.lexsort((src, gblock * 2 + parity))
    gb_s = gblock[order]
    par_s = parity[order]
    idx_s = (src[order] >> 1).astype(np.int16)
    rel_s = (dst[order] & 127).astype(np.float32)

    cnt = np.bincount(gb_s * 2 + par_s, minlength=2 * n_blocks_tot).reshape(-1, 2)
    KE = max(1, -(-int(cnt[:, 0].max()) // CHUNK))
    KO = max(1, -(-int(cnt[:, 1].max()) // CHUNK))
    NCH = KE + KO

    # block -> (core, slot) assignment, largest blocks on warm slots
    ranks = np.argsort(-(cnt[:, 0] + cnt[:, 1]), kind="stable")
    assign = ranks.reshape(NB, N_CORES)  # assign[s, c] = global block
    nregE = np.full(NB, KE * CHUNK, dtype=np.int64)
    nregO = np.full(NB, KO * CHUNK, dtype=np.int64)

    capE, capO = KE * CHUNK, KO * CHUNK
    starts = np.zeros(2 * n_blocks_tot + 1, dtype=np.int64)
    np.cumsum(cnt.reshape(-1), out=starts[1:])
    pos = np.arange(E, dtype=np.int64) - starts[gb_s * 2 + par_s]

    idx_pad = np.full((n_blocks_tot, NCH * CHUNK), pad_idx, dtype=np.int16)
    rel_pad = np.full((n_blocks_tot, NCH * CHUNK), -1.0, dtype=np.float32)
    slot = pos + par_s * capE
    idx_pad[gb_s, slot] = idx_s
    rel_pad[gb_s, slot] = rel_s

    # idx: wrapped [16, n/16] per (block, parity), replicated to all 8
    # partition groups (each gpsimd Q7 core reads its own group of 16)
    idx16 = np.empty((n_blocks_tot, 16, NCH * 8), dtype=np.int16)
    idx16[:, :, : KE * 8] = (
        idx_pad[:, :capE].reshape(n_blocks_tot, capE // 16, 16).transpose(0, 2, 1)
    )
    idx16[:, :, KE * 8 :] = (
        idx_pad[:, capE:].reshape(n_blocks_tot, capO // 16, 16).transpose(0, 2, 1)
    )
    idx_in = np.tile(idx16, (1, 8, 1))

    # relp = rel + 0.5 (staircase threshold), position-major [128, NCH],
    # each value DUPLICATED along the last axis (keeps DVE in 2x mode)
    relp_in = (rel_pad + 0.5).reshape(n_blocks_tot, NCH, CHUNK).transpose(0, 2, 1)
    relp_in = np.repeat(relp_in, 2, axis=2).astype(ml_dtypes.bfloat16)

    w1T = np.ascontiguousarray(
        W_upd[:, :D].T.astype(np.float32).astype(ml_dtypes.bfloat16)
    )
    wc = (W_upd[:, D:].astype(np.float64) @ W_msg.astype(np.float64)).astype(
        np.float32
    )
    wcT = np.ascontiguousarray(wc.T.astype(ml_dtypes.bfloat16))
    bias = np.ascontiguousarray(b_upd.astype(np.float32).reshape(P, 1))
    iota = np.ascontiguousarray(
        np.tile(np.arange(W130, dtype=np.float32), (P, 1)).astype(ml_dtypes.bfloat16)
    )

    hbf = h.astype(ml_dtypes.bfloat16)
    in_maps = []
    for c in range(N_CORES):
        blocks = assign[:, c]  # global block id per slot
        # hsT: node features for this core's assigned blocks, slot order
        hs = np.zeros((NB, P, D), dtype=ml_dtypes.bfloat16)
        for s, gb in enumerate(blocks):
            lo = gb * P
            hi = min(lo + P, N0)
            if hi > lo:
                hs[s, : hi - lo] = hbf[lo:hi]
        in_maps.append(
            {
                "h": hg,
                "hsT": np.ascontiguousarray(hs.reshape(SP, D).T),
                "idx": np.ascontiguousarray(
                    idx_in[blocks].transpose(1, 0, 2).reshape(P, NB * NCH * 8)
                ),
                "relp": np.ascontiguousarray(
                    relp_in[blocks].transpose(1, 0, 2).reshape(P, NB * NCH * 2)
                ),
                "iota": iota,
                "w1T": w1T,
                "wcT": wcT,
                "bias": bias,
            }
        )
    nreg = list(zip(nregE.tolist(), nregO.tolist()))
    return in_maps, N, SP, NB, KE, KO, nreg, assign


def kernel_with_results(h, edge_index, W_msg, W_upd, b_upd, loop_iters=None, **run_kwargs):
    in_maps, N, SP, NB, KE, KO, nreg, assign = _prep_inputs(
        h, edge_index, W_msg, W_upd, b_upd
    )

    key = (N, SP, NB, KE, KO, tuple(nreg), loop_iters)
    if key not in _prog_cache:
        _prog_cache[key] = _build_program(
            N, SP, NB, KE, KO, nreg=nreg, loop_iters=loop_iters
        )
    nc = _prog_cache[key]

    res = run_bass_kernel_spmd(nc, in_maps, core_ids=list(range(N_CORES)), **run_kwargs)

    N0 = N - 2
    out = np.empty((N0, D), dtype=np.float32)
    for c in range(N_CORES):
        oT = res.results[c]["outT"]  # [P, SP] fp32, slot-major columns
        for s in range(NB):
            gb = int(assign[s, c])
            lo = gb * P
            hi = min(lo + P, N0)
            if hi > lo:
                out[lo:hi] = oT[:, s * P : s * P + (hi - lo)].T
    return out, res


def kernel(h, edge_index, W_msg, W_upd, b_upd):
    out, _ = kernel_with_results(h, edge_index, W_msg, W_upd, b_upd)
    return out



# revision 18
# speedup vs baseline: 1.2650x; 1.2650x over previous
"""Trainium2 Bass kernel for a GNN message-passing layer.

Reference computation (all fp32):
    messages = h[src] @ W_msg.T            # [E, D]
    agg      = segment_sum(messages, dst)  # [N, D]
    out      = relu(concat(h, agg) @ W_upd.T + b_upd)

Key algebraic restructure: segment_sum is linear, so
    agg = A @ W_msg.T          where A = segment_sum(h[src], dst)
and the update splits W_upd = [Wu1 | Wu2]:
    out.T = relu(Wu1 @ h.T + (Wu2 @ W_msg) @ A.T + b)
so the device only computes A (a pure gather + scatter-add) plus two small
fused matmuls.  Wc = Wu2 @ W_msg is precomputed on host.

Sharding: nodes are partitioned contiguously across the 8 cores by dst.
Each core processes exactly the edges whose dst lands in its node shard,
so no collectives are needed.

The kernel is SWDGE-bound: HW-measured cost ~ 1.1 ns per descriptor
+ ~0.93 ns per 256B of ring payload + ~0.46 us per gather instruction.
To cut descriptor count, each descriptor fetches a 512B WINDOW = two
adjacent bf16 rows of a permuted copy of the h table.  The host builds
L=6 per-core permutations, each realizing a disjoint node *matching*
chosen so that two edges of the same dst-block share one window (~88%
of edges pair up); the rest use one half of a window (the unused half's
staircase rel is -1, an all-zero staircase row, contributing 0).
Window indices fit int16 (25001 windows incl. one zero pad window).

Aggregation per 128-descriptor chunk (slot i -> partition i%128):
  S[slot, jj] = (jj < rel[slot] + 0.5)   one DVE compare per
  (chunk, half, block-section piece); then on TensorE (bf16):
  psum_blk[feat, jj] += g_half[slot, feat] * S[slot, jj]; per-dst sums
  are adjacent-column diffs of psum.  relp is stored duplicated (each
  value twice) to keep the DVE in its 2x 16-bit mode.
Gather instructions cover 4 dst-blocks x 1 copy each (static
per-(group,copy,block) section capacities = max over the 8 cores, so
the single SPMD program works for every core's data).
Phase 2 (per 4-block group): diff on VectorE (fp32 -> bf16), then
    out.T = relu(Wu1 @ h.T + Wc @ diff + b)   (bf16 matmuls)
"""

import contextlib

import numpy as np

import concourse.bass as bass
import concourse.mybir as mybir
import concourse.tile as tile
from concourse import bacc
from concourse.bass_utils import run_bass_kernel_spmd

P = 128  # SBUF partitions
D = 128  # feature dim (in_dim == out_dim == 128)
N_CORES = 8
CHUNK = 128  # descriptors per matmul chunk
W129 = CHUNK + 1  # staircase width per block (psum / buf)
W130 = CHUNK + 2  # staircase width incl. pad col (even for 2x DVE mode)
L_COPIES = 6  # permuted pair-table copies per core
GB = 4  # dst-blocks per gather group

_prog_cache: dict = {}


def _build_program(plan, loop_iters=None):
    """One SPMD program, shared by all 8 cores; static sizes from `plan`."""
    f32 = mybir.dt.float32
    bf16 = mybir.dt.bfloat16
    i16 = mybir.dt.int16
    NB = plan["NB"]
    SP = NB * P
    NW = plan["NW"]  # windows per table (incl. zero pad window)
    caps = plan["caps"]  # caps[g][k]: stream descriptor count (mult of 128)
    pieces = plan["pieces"]  # pieces[g][k]: list of (chunk, half, bi, start, stop)
    npieces = plan["npieces"]
    groups = plan["groups"]  # list of (b0, nb)
    idx_cols = plan["idx_cols"]
    L = len(caps[0])

    nc = bacc.Bacc("TRN2", target_bir_lowering=False, num_swdge_queues=4)

    tabs_d = [
        nc.dram_tensor(f"tab{k}", [NW, 2 * D], bf16, kind="ExternalInput")
        for k in range(L)
    ]
    hsT_d = nc.dram_tensor("hsT", [P, SP], bf16, kind="ExternalInput")
    idx_d = nc.dram_tensor("idx", [P, idx_cols], i16, kind="ExternalInput")
    relp_d = nc.dram_tensor("relp", [P, npieces * 2], bf16, kind="ExternalInput")
    iota_d = nc.dram_tensor("iota", [P, W130], bf16, kind="ExternalInput")
    w1_d = nc.dram_tensor("w1T", [D, D], bf16, kind="ExternalInput")
    wc_d = nc.dram_tensor("wcT", [D, D], bf16, kind="ExternalInput")
    b_d = nc.dram_tensor("bias", [P, 1], f32, kind="ExternalInput")
    out_d = nc.dram_tensor("outT", [P, SP], f32, kind="ExternalOutput")

    capmax = max(max(ck) for ck in caps)

    with tile.TileContext(nc) as tc:
        with (
            tc.tile_pool(name="constp", bufs=1) as constp,
            tc.tile_pool(name="gatp", bufs=8) as gatp,
            tc.tile_pool(name="sp_", bufs=4) as sp_,
            tc.tile_pool(name="aggp", bufs=1) as aggp,
            tc.tile_pool(name="diffp", bufs=2) as diffp,
            tc.tile_pool(name="outp", bufs=3) as outp,
            tc.tile_pool(name="psp", bufs=6, space="PSUM") as psp,
            tc.tile_pool(name="ps2p", bufs=2, space="PSUM") as ps2p,
        ):
            iota_t = constp.tile([P, W130], bf16)
            nc.sync.dma_start(iota_t[:], iota_d[:])
            # idx split so the first gathers start immediately
            idx_t = constp.tile([P, idx_cols], i16)
            c1 = min(caps[0][0] // 16, idx_cols)
            nc.sync.dma_start(idx_t[:, 0:c1], idx_d[:, 0:c1])
            c2 = min(sum(caps[0]) // 16, idx_cols)
            if c2 > c1:
                nc.sync.dma_start(idx_t[:, c1:c2], idx_d[:, c1:c2])
            if idx_cols > c2:
                nc.sync.dma_start(idx_t[:, c2:], idx_d[:, c2:])
            relp_t = constp.tile([P, npieces * 2], bf16)
            nc.sync.dma_start(relp_t[:], relp_d[:])
            w1_t = constp.tile([D, D], bf16)
            nc.sync.dma_start(w1_t[:], w1_d[:])
            wc_t = constp.tile([D, D], bf16)
            nc.sync.dma_start(wc_t[:], wc_d[:])
            b_t = constp.tile([P, 1], f32)
            nc.sync.dma_start(b_t[:], b_d[:])
            hsT_t = constp.tile([P, SP], bf16)
            nc.sync.dma_start(hsT_t[:], hsT_d[:])

            buf_t = aggp.tile([P, NB * W129], f32)
            iota_ab = iota_t[:].rearrange("p (a b) -> p a b", b=2)

            loop_cm = (
                tc.For_i(0, loop_iters, 1)
                if loop_iters is not None
                else contextlib.nullcontext()
            )
            with loop_cm:
                icol = 0
                ipiece = 0
                qn = 0
                for g, (b0g, nbg) in enumerate(groups):
                    ps_ts = [
                        psp.tile([P, W129], f32, name="ps_t") for bi in range(nbg)
                    ]
                    for k in range(L):
                        cap = caps[g][k]
                        pl = pieces[g][k]
                        np_k = len(pl)
                        g_t = gatp.tile([P, capmax * 2], bf16)
                        g3 = g_t[:].rearrange("p (c d) -> p c d", d=2 * D)
                        nc.gpsimd.dma_gather(
                            out_ap=g3[:, 0 : cap // 128, :],
                            in_ap=tabs_d[k][:],
                            idxs_ap=idx_t[:, icol : icol + cap // 16],
                            num_idxs=cap,
                            num_idxs_reg=cap,
                            elem_size=2 * D,
                            single_packet=False,
                            queue_num=qn,
                        )
                        icol += cap // 16
                        qn = (qn + 1) % 4
                        if np_k == 0:
                            continue
                        s_t = sp_.tile([P, np_k * W130], bf16)
                        relp_b = (
                            relp_t[:, ipiece * 2 : (ipiece + np_k) * 2]
                            .rearrange("p (c t) -> p c t", t=2)
                            .unsqueeze(2)
                            .broadcast_to([P, np_k, W130 // 2, 2])
                        )
                        iota_b = iota_ab.unsqueeze(1).broadcast_to(
                            [P, np_k, W130 // 2, 2]
                        )
                        s_b = s_t[:].rearrange(
                            "p (c a b) -> p c a b", a=W130 // 2, b=2
                        )
                        nc.vector.tensor_tensor(
                            out=s_b, in0=iota_b, in1=relp_b, op=mybir.AluOpType.is_lt
                        )
                        s3 = s_t[:].rearrange("p (c w) -> p c w", w=W130)
                        for pi, (ch, half, bi, start, stop) in enumerate(pl):
                            nc.tensor.matmul(
                                out=ps_ts[bi][:],
                                lhsT=g_t[
                                    :,
                                    ch * 2 * D + half * D : ch * 2 * D + half * D + D,
                                ],
                                rhs=s3[:, pi, 0:W129],
                                start=bool(start),
                                stop=bool(stop),
                            )
                        ipiece += np_k
                    for bi in range(nbg):
                        nc.scalar.activation(
                            out=buf_t[:, (b0g + bi) * W129 : (b0g + bi + 1) * W129],
                            in_=ps_ts[bi][:],
                            func=mybir.ActivationFunctionType.Copy,
                        )

                # Phase 2 per 4-block group
                buf3 = buf_t[:].rearrange("p (b j) -> p b j", j=W129)
                b0 = 0
                while b0 < NB:
                    nb = min(4, NB - b0)
                    w = nb * CHUNK
                    col = b0 * CHUNK
                    d_t = diffp.tile([P, 512], bf16)
                    d3 = d_t[:].rearrange("p (b j) -> p b j", j=CHUNK)
                    nc.vector.tensor_tensor(
                        out=d3[:, 0:nb, :],
                        in0=buf3[:, b0 : b0 + nb, 0:CHUNK],
                        in1=buf3[:, b0 : b0 + nb, 1:W129],
                        op=mybir.AluOpType.subtract,
                    )
                    ps2_t = ps2p.tile([P, 512], f32)
                    nc.tensor.matmul(
                        out=ps2_t[:, :w],
                        lhsT=w1_t[:],
                        rhs=hsT_t[:, col : col + w],
                        start=True,
                        stop=False,
                    )
                    nc.tensor.matmul(
                        out=ps2_t[:, :w],
                        lhsT=wc_t[:],
                        rhs=d_t[:, :w],
                        start=False,
                        stop=True,
                    )
                    o_t = outp.tile([P, 512], f32)
                    nc.scalar.activation(
                        o_t[:, :w],
                        ps2_t[:, :w],
                        mybir.ActivationFunctionType.Relu,
                        bias=b_t[:],
                    )
                    nc.sync.dma_start(out_d[:, col : col + w], o_t[:, :w])
                    b0 += nb

    nc.compile()
    return nc


def _prep_inputs(h, edge_index, W_msg, W_upd, b_upd):
    """Host prep: per-core edge bucketing, L matchings -> pair tables,
    static stream/piece plan shared across cores."""
    import ml_dtypes

    N0, d = h.shape
    assert d == D
    E = edge_index.shape[1]

    SP = -(-N0 // (N_CORES * P)) * P
    NB = SP // P
    NWIN = N0 // 2
    PADW = NWIN  # zero pad window index
    NW = NWIN + 1
    L = L_COPIES

    src = np.ascontiguousarray(edge_index[0]).astype(np.int64)
    dst = np.ascontiguousarray(edge_index[1]).astype(np.int64)
    gblock = dst >> 7
    core_of = np.minimum(gblock // NB, N_CORES - 1)
    slot_of = gblock - core_of * NB
    rel = (dst & 127).astype(np.int64)

    groups = []
    b0 = 0
    while b0 < NB:
        nb = min(GB, NB - b0)
        groups.append((b0, nb))
        b0 += nb
    NG = len(groups)
    group_of_slot = np.zeros(NB, np.int64)
    for gi, (b0g, nbg) in enumerate(groups):
        group_of_slot[b0g : b0g + nbg] = gi

    rng = np.random.default_rng(12345)
    hbf = h.astype(ml_dtypes.bfloat16)

    # entries[c][g][k][slot] = list of (window, relA, relB)
    entries = [
        [[[[] for _ in range(NB)] for _ in range(L)] for _ in range(NG)]
        for _ in range(N_CORES)
    ]
    tables = [[None] * L for _ in range(N_CORES)]

    for c in range(N_CORES):
        m = np.flatnonzero(core_of == c)
        es = src[m]
        eslot = slot_of[m]
        erel = rel[m]
        covered = np.zeros(len(es), bool)
        pos_maps = []
        for k in range(L):
            ridx = np.flatnonzero(~covered)
            o = np.lexsort((es[ridx], eslot[ridx]))
            r = ridx[o]
            rb = eslot[r]
            same = rb[:-1] == rb[1:]
            newrun = np.r_[True, rb[1:] != rb[:-1]]
            pos = np.arange(len(r)) - np.maximum.accumulate(
                np.where(newrun, np.arange(len(r)), 0)
            )
            i_idx = np.flatnonzero((pos[:-1] % 2 == 0) & same)
            e1 = r[i_idx]
            e2 = r[i_idx + 1]
            s1 = es[e1]
            s2 = es[e2]
            v = s1 != s2
            e1, e2, s1, s2 = e1[v], e2[v], s1[v], s2[v]
            taken = np.zeros(N0, bool)
            acc_e1, acc_e2 = [], []
            remaining = rng.permutation(len(e1))
            for _ in range(4):
                if len(remaining) == 0:
                    break
                aa, bb = s1[remaining], s2[remaining]
                ok = ~taken[aa] & ~taken[bb]
                cand = remaining[ok]
                if len(cand) == 0:
                    break
                aa, bb = s1[cand], s2[cand]
                fa = np.zeros(len(cand), bool)
                fb = np.zeros(len(cand), bool)
                _, fi = np.unique(aa, return_index=True)
                fa[fi] = True
                _, fi2 = np.unique(bb, return_index=True)
                fb[fi2] = True
                acc = np.flatnonzero(fa & fb)
                s1a, s2a = aa[acc], bb[acc]
                seen2 = np.zeros(N0, bool)
                keep = np.zeros(len(acc), bool)
                for i_ in range(len(acc)):
                    x, y = s1a[i_], s2a[i_]
                    if not (seen2[x] or seen2[y]):
                        keep[i_] = True
                        seen2[x] = True
                        seen2[y] = True
                accepted = cand[acc[keep]]
                taken[s1[accepted]] = True
                taken[s2[accepted]] = True
                acc_e1.append(e1[accepted])
                acc_e2.append(e2[accepted])
                covered[e1[accepted]] = True
                covered[e2[accepted]] = True
                remaining = remaining[~taken[s1[remaining]] & ~taken[s2[remaining]]]
            pe1 = np.concatenate(acc_e1) if acc_e1 else np.empty(0, np.int64)
            pe2 = np.concatenate(acc_e2) if acc_e2 else np.empty(0, np.int64)
            ps1, ps2 = es[pe1], es[pe2]
            used = np.zeros(N0, bool)
            used[ps1] = True
            used[ps2] = True
            restn = np.flatnonzero(~used)
            perm = np.empty(N0, np.int64)
            npair = len(ps1)
            perm[0 : 2 * npair : 2] = ps1
            perm[1 : 2 * npair : 2] = ps2
            perm[2 * npair :] = restn
            pos_of = np.empty(N0, np.int64)
            pos_of[perm] = np.arange(N0)
            pos_maps.append(pos_of)
            tab = np.zeros((NW, 2 * D), dtype=ml_dtypes.bfloat16)
            tab[:NWIN] = hbf[perm].reshape(NWIN, 2 * D)
            tables[c][k] = tab
            sl_arr = eslot[pe1]
            ra_arr = erel[pe1]
            rb_arr = erel[pe2]
            for j in range(npair):
                sl = int(sl_arr[j])
                entries[c][group_of_slot[sl]][k][sl].append(
                    (j, int(ra_arr[j]), int(rb_arr[j]))
                )
        sing = np.flatnonzero(~covered)
        for j, ei in enumerate(sing):
            k = j % L
            p = int(pos_maps[k][es[ei]])
            w = p >> 1
            sl = int(eslot[ei])
            if p & 1 == 0:
                entries[c][group_of_slot[sl]][k][sl].append((w, int(erel[ei]), -1))
            else:
                entries[c][group_of_slot[sl]][k][sl].append((w, -1, int(erel[ei])))

    # static caps per (g, k, block): max over cores, mult of 16
    capgkb = np.zeros((NG, L, NB), np.int64)
    for g in range(NG):
        for k in range(L):
            for bsl in range(NB):
                mx = max(len(entries[c][g][k][bsl]) for c in range(N_CORES))
                capgkb[g, k, bsl] = -(-mx // 16) * 16
    caps = []
    for g in range(NG):
        ck = []
        for k in range(L):
            t = int(capgkb[g, k, :].sum())
            ck.append(-(-max(t, 128) // 128) * 128)
        caps.append(ck)

    # static sections and piece lists
    stream_sections = [[None] * L for _ in range(NG)]
    pieces = [[None] * L for _ in range(NG)]
    for g, (b0g, nbg) in enumerate(groups):
        for k in range(L):
            secs = []
            off = 0
            for bi in range(nbg):
                sl = b0g + bi
                cb = int(capgkb[g, k, sl])
                if cb:
                    secs.append((off, off + cb, bi, sl))
                off += cb
            stream_sections[g][k] = secs
            cap = caps[g][k]
            pl = []
            for ch in range(cap // 128):
                lo_c, hi_c = ch * 128, (ch + 1) * 128
                for off_lo, off_hi, bi, sl in secs:
                    a = max(lo_c, off_lo)
                    b_ = min(hi_c, off_hi)
                    if a < b_:
                        for half in (0, 1):
                            pl.append([ch, half, bi, 0, 0, a - lo_c, b_ - lo_c, sl])
            pieces[g][k] = pl
    # start/stop flags in program emission order
    seen_first = set()
    last_ref = {}
    for g in range(NG):
        for k in range(L):
            for p in pieces[g][k]:
                sl = p[7]
                if sl not in seen_first:
                    p[3] = 1
                    seen_first.add(sl)
                last_ref[sl] = p
    for sl, p in last_ref.items():
        p[4] = 1
    assert len(seen_first) == NB, (len(seen_first), NB)

    npieces = sum(len(pieces[g][k]) for g in range(NG) for k in range(L))
    idx_cols = sum(caps[g][k] // 16 for g in range(NG) for k in range(L))

    w1T = np.ascontiguousarray(
        W_upd[:, :D].T.astype(np.float32).astype(ml_dtypes.bfloat16)
    )
    wc = (W_upd[:, D:].astype(np.float64) @ W_msg.astype(np.float64)).astype(
        np.float32
    )
    wcT = np.ascontiguousarray(wc.T.astype(ml_dtypes.bfloat16))
    bias = np.ascontiguousarray(b_upd.astype(np.float32).reshape(P, 1))
    iota = np.ascontiguousarray(
        np.tile(np.arange(W130, dtype=np.float32), (P, 1)).astype(ml_dtypes.bfloat16)
    )

    in_maps = []
    for c in range(N_CORES):
        idx_flat = np.full(idx_cols * 16, PADW, np.int16)
        relp_arr = np.full((P, npieces * 2), -0.5, np.float32)
        ioff = 0
        poff = 0
        for g, (b0g, nbg) in enumerate(groups):
            for k in range(L):
                cap = caps[g][k]
                secs = stream_sections[g][k]
                slots_w = np.full(cap, PADW, np.int64)
                slots_rA = np.full(cap, -1.0, np.float32)
                slots_rB = np.full(cap, -1.0, np.float32)
                for off_lo, off_hi, bi, sl in secs:
                    ent = entries[c][g][k][sl]
                    n = len(ent)
                    if n:
                        slots_w[off_lo : off_lo + n] = [e[0] for e in ent]
                        slots_rA[off_lo : off_lo + n] = [e[1] for e in ent]
                        slots_rB[off_lo : off_lo + n] = [e[2] for e in ent]
                idx_flat[ioff : ioff + cap] = slots_w.astype(np.int16)
                ioff += cap
                for p in pieces[g][k]:
                    ch, half, bi, st, sp2, a, b_, sl = p
                    vals = np.full(P, -0.5, np.float32)
                    base = ch * 128
                    rr = (slots_rA if half == 0 else slots_rB)[base + a : base + b_]
                    vals[a:b_] = rr + 0.5
                    relp_arr[:, poff * 2] = vals
                    relp_arr[:, poff * 2 + 1] = vals
                    poff += 1
        assert ioff == idx_cols * 16 and poff == npieces
        idx16 = idx_flat.reshape(idx_cols, 16).T  # [16, cols]
        idx_in = np.tile(idx16, (8, 1))

        lo = c * SP
        hi = min((c + 1) * SP, N0)
        hs = np.zeros((SP, D), dtype=ml_dtypes.bfloat16)
        if hi > lo:
            hs[: hi - lo] = hbf[lo:hi]
        im = {
            "hsT": np.ascontiguousarray(hs.T),
            "idx": np.ascontiguousarray(idx_in),
            "relp": np.ascontiguousarray(relp_arr.astype(ml_dtypes.bfloat16)),
            "iota": iota,
            "w1T": w1T,
            "wcT": wcT,
            "bias": bias,
        }
        for k in range(L):
            im[f"tab{k}"] = tables[c][k]
        in_maps.append(im)

    plan = {
        "NB": NB,
        "NW": NW,
        "caps": caps,
        "pieces": [
            [[(p[0], p[1], p[2], p[3], p[4]) for p in pieces[g][k]] for k in range(L)]
            for g in range(NG)
        ],
        "npieces": npieces,
        "groups": groups,
        "idx_cols": idx_cols,
    }
    return in_maps, plan, SP, NB


def kernel_with_results(h, edge_index, W_msg, W_upd, b_upd, loop_iters=None, **run_kwargs):
    in_maps, plan, SP, NB = _prep_inputs(h, edge_index, W_msg, W_upd, b_upd)

    key = (
        plan["NB"],
        plan["NW"],
        tuple(tuple(ck) for ck in plan["caps"]),
        tuple(
            tuple(map(tuple, plan["pieces"][g][k]))
            for g in range(len(plan["caps"]))
            for k in range(L_COPIES)
        ),
        loop_iters,
    )
    if key not in _prog_cache:
        _prog_cache[key] = _build_program(plan, loop_iters=loop_iters)
    nc = _prog_cache[key]

    res = run_bass_kernel_spmd(nc, in_maps, core_ids=list(range(N_CORES)), **run_kwargs)

    N0 = h.shape[0]
    out = np.empty((N0, D), dtype=np.float32)
    for c in range(N_CORES):
        lo = c * SP
        hi = min((c + 1) * SP, N0)
        if hi > lo:
            out[lo:hi] = res.results[c]["outT"].T[: hi - lo]
    return out, res


def kernel(h, edge_index, W_msg, W_upd, b_upd):
    out, _ = kernel_with_results(h, edge_index, W_msg, W_upd, b_upd)
    return out
